# revision 8
# baseline (speedup 1.0000x reference)
"""Trainium2 Bass kernel for the Mahalanobis loss:

    out = mean_b( sqrt( delta[b] @ S_inv @ delta[b] ) ),  delta = original - reconstruction

Full shapes: original/reconstruction [8192, 2048] f32, S_inv [2048, 2048] f32.

Strategy (data-parallel over batch, 8 NeuronCores):
  - Core i handles rows [i*1024, (i+1)*1024). S_inv replicated.
  - Per core: delta computed on DVE (f32 sub -> bf16 out), transposed to
    [d, b] layout via DMA-transpose (bf16), S_inv cast to bf16 (ACT).
  - Y = delta @ S_inv as 128x128 stationary (delta^T tiles) x [128, 512]
    moving (S) bf16 matmuls accumulated f32 in PSUM over 16 K-blocks.
  - q[b] = rowsum(delta_bf16 * Y) fused in one DVE tensor_tensor_reduce per
    (b_tile, e_chunk), chain-accumulated into q_all[:, b_tile].
  - Per-core output: q_out [128, 8] f32 (q for its 1024 rows).
  - Host: concat shards, sqrt, mean  (exact f64 host math, cast to f32).

Numerics: bf16 matmul with f32 accumulation gives ~5e-5 relative error on the
final scalar (validated against f64 numpy).
"""

import numpy as np

P = 128
B_FULL, D = 8192, 2048
N_CORES = 8
B_SH = B_FULL // N_CORES  # 1024
EC = 512                  # matmul moving free dim / PSUM bank (f32)

_CACHED = {}


def _build(b_sh=B_SH, d=D):
    import concourse.tile as tile
    from concourse import bacc, mybir

    NB = b_sh // P   # batch tiles per core
    NJ = d // P      # contraction K-blocks
    NE = d // EC     # e-chunks (output columns / 512)

    # Bacc (not raw Bass): its compile() legalizes semaphore waits
    # (move_matmul_waits_to_ldweights + generate_event_semaphores) — TRN2
    # instructions can embed only ONE sync wait.
    nc = bacc.Bacc("TRN2", target_bir_lowering=False)
    f32 = mybir.dt.float32
    bf16 = mybir.dt.bfloat16

    orig = nc.dram_tensor("orig", [b_sh, d], f32, kind="ExternalInput")
    recon = nc.dram_tensor("recon", [b_sh, d], f32, kind="ExternalInput")
    s_inv = nc.dram_tensor("s_inv", [d, d], f32, kind="ExternalInput")
    q_out = nc.dram_tensor("q_out", [P, NB], f32, kind="ExternalOutput")

    with tile.TileContext(nc) as tc:
        with (
            tc.tile_pool(name="io", bufs=2) as io_pool,
            tc.tile_pool(name="sstage", bufs=6) as s_stage,
            tc.tile_pool(name="sbf", bufs=1) as s_pool,
            tc.tile_pool(name="dbf", bufs=1) as d_pool,
            tc.tile_pool(name="dT", bufs=1) as dT_pool,
            tc.tile_pool(name="scr", bufs=2) as scr_pool,
            tc.tile_pool(name="qp", bufs=1) as q_pool,
            tc.tile_pool(name="psum", bufs=4, space="PSUM") as psum_pool,
        ):
            q_all = q_pool.tile([P, NB], f32, name="q_all", tag="q_all")
            q_part = q_pool.tile([P, NB, NE], f32, name="q_part", tag="q_part")
            delta_bf = [None] * NB
            deltaT = [None] * NB
            s_bf = [[None] * NE for _ in range(NJ)]

            for e in range(NE):
                for t in range(NB):
                    if e == 0:
                        # delta pipeline for batch tile t.
                        # Plain loads go on the ACT (scalar) HWDGE queue so the
                        # SP queue carries only transposes: a transpose waits on
                        # the DVE subtract, and an in-order DMA queue would
                        # stall every later load behind that wait.
                        o_t = io_pool.tile([P, d], f32, name=f"o_{t}", tag="o")
                        nc.scalar.dma_start(o_t[:], orig[t * P:(t + 1) * P, :])
                        r_t = io_pool.tile([P, d], f32, name=f"r_{t}", tag="r")
                        nc.scalar.dma_start(r_t[:], recon[t * P:(t + 1) * P, :])
                        db = d_pool.tile([P, d], bf16, name=f"dbf_{t}",
                                         tag=f"dbf_{t}")
                        nc.vector.tensor_sub(db[:], o_t[:], r_t[:])
                        dT = dT_pool.tile([P, NJ, P], bf16, name=f"dT_{t}",
                                          tag=f"dT_{t}")
                        # dT[p, j, b] = db[b, j*128 + p]  (verified in CoreSim)
                        nc.sync.dma_start(dT[:], db[:], transpose=True)
                        delta_bf[t] = db
                        deltaT[t] = dT

                    ps = psum_pool.tile([P, EC], f32, name=f"ps_{e}_{t}",
                                        tag="ps")
                    for j in range(NJ):
                        if t == 0:
                            # S piece (j, e): f32 load + ACT cast to bf16
                            sf = s_stage.tile([P, EC], f32, name=f"sf_{j}_{e}",
                                              tag="sf")
                            nc.sync.dma_start(
                                sf[:],
                                s_inv[j * P:(j + 1) * P, e * EC:(e + 1) * EC])
                            sb = s_pool.tile([P, EC], bf16, name=f"s_{j}_{e}",
                                             tag=f"s_{j}_{e}")
                            nc.scalar.copy(sb[:], sf[:])
                            s_bf[j][e] = sb
                        nc.tensor.matmul(
                            ps[:],
                            deltaT[t][:, j, :],
                            s_bf[j][e][:],
                            start=(j == 0),
                            stop=(j == NJ - 1),
                        )
                    # q-partial: product then row-reduce (two plain DVE ops;
                    # tensor_tensor_reduce faults the device on this runtime)
                    scr = scr_pool.tile([P, EC], f32, name=f"scr_{e}_{t}",
                                        tag="scr")
                    nc.vector.tensor_tensor(
                        scr[:], ps[:], delta_bf[t][:, e * EC:(e + 1) * EC],
                        mybir.AluOpType.mult)
                    nc.vector.tensor_reduce(
                        out=q_part[:, t, e:e + 1], in_=scr[:],
                        axis=mybir.AxisListType.X, op=mybir.AluOpType.add)

            nc.vector.tensor_reduce(out=q_all[:, :, None], in_=q_part[:],
                                    axis=mybir.AxisListType.X,
                                    op=mybir.AluOpType.add)
            nc.sync.dma_start(q_out[:], q_all[:])

    nc.compile()
    return nc


def _get_nc():
    if "nc" not in _CACHED:
        _CACHED["nc"] = _build()
    return _CACHED["nc"]


def kernel(original: np.ndarray, reconstruction: np.ndarray,
           S_inv: np.ndarray) -> np.ndarray:
    from concourse import bass_utils

    nc = _get_nc()
    s_full = np.ascontiguousarray(np.asarray(S_inv, dtype=np.float32))
    in_maps = []
    for i in range(N_CORES):
        sl = slice(i * B_SH, (i + 1) * B_SH)
        in_maps.append({
            "orig": np.ascontiguousarray(np.asarray(original[sl], np.float32)),
            "recon": np.ascontiguousarray(
                np.asarray(reconstruction[sl], np.float32)),
            "s_inv": s_full,
        })

    res = bass_utils.run_bass_kernel_spmd(
        nc, in_maps, core_ids=list(range(N_CORES)),
        trace=_CACHED.get("trace", False),
    )
    _CACHED["last_results"] = res

    q = np.concatenate(
        [np.asarray(r["q_out"]).T.reshape(-1) for r in res.results])
    out = np.sqrt(q.astype(np.float64)).mean()
    return np.float32(out)


# revision 10
# speedup vs baseline: 15.2687x; 15.2687x over previous
"""Trainium2 Bass kernel for the Mahalanobis loss:

    out = mean_b( sqrt( delta[b] @ S_inv @ delta[b] ) ),  delta = original - reconstruction

Full shapes: original/reconstruction [8192, 2048] f32, S_inv [2048, 2048] f32.

Strategy (data-parallel over batch, 8 NeuronCores):
  - Core i handles rows [i*1024, (i+1)*1024). S_inv replicated.
  - Per core: delta computed on DVE (f32 sub -> bf16 out), transposed to
    [d, b] layout via DMA-transpose (bf16), S_inv cast to bf16 (ACT).
  - Y = delta @ S_inv as 128x128 stationary (delta^T tiles) x [128, 512]
    moving (S) bf16 matmuls accumulated f32 in PSUM over 16 K-blocks.
  - q[b] = rowsum(delta_bf16 * Y) fused in one DVE tensor_tensor_reduce per
    (b_tile, e_chunk), chain-accumulated into q_all[:, b_tile].
  - Per-core output: q_out [128, 8] f32 (q for its 1024 rows).
  - Host: concat shards, sqrt, mean  (exact f64 host math, cast to f32).

Numerics: bf16 matmul with f32 accumulation gives ~5e-5 relative error on the
final scalar (validated against f64 numpy).
"""

import numpy as np

P = 128
B_FULL, D = 8192, 2048
N_CORES = 8
B_SH = B_FULL // N_CORES  # 1024
EC = 512                  # matmul moving free dim / PSUM bank (f32)

_CACHED = {}


def _build(b_sh=B_SH, d=D, loop=1):
    import contextlib

    import concourse.tile as tile
    from concourse import bacc, mybir

    NB = b_sh // P   # batch tiles per core
    NJ = d // P      # contraction K-blocks
    NE = d // EC     # e-chunks (output columns / 512)

    # Bacc (not raw Bass): its compile() legalizes semaphore waits
    # (move_matmul_waits_to_ldweights + generate_event_semaphores) — TRN2
    # instructions can embed only ONE sync wait.
    nc = bacc.Bacc("TRN2", target_bir_lowering=False)
    f32 = mybir.dt.float32
    bf16 = mybir.dt.bfloat16

    orig = nc.dram_tensor("orig", [b_sh, d], f32, kind="ExternalInput")
    recon = nc.dram_tensor("recon", [b_sh, d], f32, kind="ExternalInput")
    s_inv = nc.dram_tensor("s_inv", [d, d], f32, kind="ExternalInput")
    q_out = nc.dram_tensor("q_out", [P, NB], f32, kind="ExternalOutput")

    with tile.TileContext(nc) as tc:
        with (
            tc.tile_pool(name="io", bufs=2) as io_pool,
            tc.tile_pool(name="sstage", bufs=6) as s_stage,
            tc.tile_pool(name="sbf", bufs=1) as s_pool,
            tc.tile_pool(name="dbf", bufs=1) as d_pool,
            tc.tile_pool(name="dT", bufs=1) as dT_pool,
            tc.tile_pool(name="scr", bufs=2) as scr_pool,
            tc.tile_pool(name="qp", bufs=1) as q_pool,
            tc.tile_pool(name="psum", bufs=4, space="PSUM") as psum_pool,
            tc.For_i(0, loop, 1) if loop > 1 else contextlib.nullcontext(),
        ):
            q_all = q_pool.tile([P, NB], f32, name="q_all", tag="q_all")
            q_part = q_pool.tile([P, NB, NE], f32, name="q_part", tag="q_part")
            delta_bf = [None] * NB
            deltaT = [None] * NB
            s_bf = [[None] * NE for _ in range(NJ)]

            for e in range(NE):
                for t in range(NB):
                    if e == 0:
                        # delta pipeline for batch tile t.
                        # Plain loads go on the ACT (scalar) HWDGE queue so the
                        # SP queue carries only transposes: a transpose waits on
                        # the DVE subtract, and an in-order DMA queue would
                        # stall every later load behind that wait.
                        o_t = io_pool.tile([P, d], f32, name=f"o_{t}", tag="o")
                        nc.scalar.dma_start(o_t[:], orig[t * P:(t + 1) * P, :])
                        r_t = io_pool.tile([P, d], f32, name=f"r_{t}", tag="r")
                        nc.scalar.dma_start(r_t[:], recon[t * P:(t + 1) * P, :])
                        db = d_pool.tile([P, d], bf16, name=f"dbf_{t}",
                                         tag=f"dbf_{t}")
                        nc.vector.tensor_sub(db[:], o_t[:], r_t[:])
                        dT = dT_pool.tile([P, NJ, P], bf16, name=f"dT_{t}",
                                          tag=f"dT_{t}")
                        # dT[p, j, b] = db[b, j*128 + p]  (verified in CoreSim)
                        nc.sync.dma_start(dT[:], db[:], transpose=True)
                        delta_bf[t] = db
                        deltaT[t] = dT

                    ps = psum_pool.tile([P, EC], f32, name=f"ps_{e}_{t}",
                                        tag="ps")
                    for j in range(NJ):
                        if t == 0:
                            # S piece (j, e): f32 load + ACT cast to bf16
                            sf = s_stage.tile([P, EC], f32, name=f"sf_{j}_{e}",
                                              tag="sf")
                            nc.sync.dma_start(
                                sf[:],
                                s_inv[j * P:(j + 1) * P, e * EC:(e + 1) * EC])
                            sb = s_pool.tile([P, EC], bf16, name=f"s_{j}_{e}",
                                             tag=f"s_{j}_{e}")
                            nc.scalar.copy(sb[:], sf[:])
                            s_bf[j][e] = sb
                        nc.tensor.matmul(
                            ps[:],
                            deltaT[t][:, j, :],
                            s_bf[j][e][:],
                            start=(j == 0),
                            stop=(j == NJ - 1),
                        )
                    # q-partial: product then row-reduce (two plain DVE ops;
                    # tensor_tensor_reduce faults the device on this runtime)
                    scr = scr_pool.tile([P, EC], f32, name=f"scr_{e}_{t}",
                                        tag="scr")
                    nc.vector.tensor_tensor(
                        scr[:], ps[:], delta_bf[t][:, e * EC:(e + 1) * EC],
                        mybir.AluOpType.mult)
                    nc.vector.tensor_reduce(
                        out=q_part[:, t, e:e + 1], in_=scr[:],
                        axis=mybir.AxisListType.X, op=mybir.AluOpType.add)

            nc.vector.tensor_reduce(out=q_all[:, :, None], in_=q_part[:],
                                    axis=mybir.AxisListType.X,
                                    op=mybir.AluOpType.add)
            nc.sync.dma_start(q_out[:], q_all[:])

    nc.compile()
    return nc


def _get_nc():
    if "nc" not in _CACHED:
        _CACHED["nc"] = _build()
    return _CACHED["nc"]


def kernel(original: np.ndarray, reconstruction: np.ndarray,
           S_inv: np.ndarray) -> np.ndarray:
    from concourse import bass_utils

    nc = _get_nc()
    s_full = np.ascontiguousarray(np.asarray(S_inv, dtype=np.float32))
    in_maps = []
    for i in range(N_CORES):
        sl = slice(i * B_SH, (i + 1) * B_SH)
        in_maps.append({
            "orig": np.ascontiguousarray(np.asarray(original[sl], np.float32)),
            "recon": np.ascontiguousarray(
                np.asarray(reconstruction[sl], np.float32)),
            "s_inv": s_full,
        })

    res = bass_utils.run_bass_kernel_spmd(
        nc, in_maps, core_ids=list(range(N_CORES)),
        trace=_CACHED.get("trace", False),
    )
    _CACHED["last_results"] = res

    q = np.concatenate(
        [np.asarray(r["q_out"]).T.reshape(-1) for r in res.results])
    out = np.sqrt(q.astype(np.float64)).mean()
    return np.float32(out)


# revision 15
# speedup vs baseline: 22.4548x; 1.4706x over previous
"""Trainium2 Bass kernel for the Mahalanobis loss:

    out = mean_b( sqrt( delta[b] @ S_inv @ delta[b] ) ),  delta = original - reconstruction

Full shapes: original/reconstruction [8192, 2048] f32, S_inv [2048, 2048] f32.

Strategy (data-parallel over batch, 8 NeuronCores):
  - Core i handles rows [i*1024, (i+1)*1024). S_inv replicated.
  - Per core: delta computed on DVE (f32 sub -> bf16 out), transposed to
    [d, b] layout via DMA-transpose (bf16), S_inv cast to bf16 (ACT).
  - Y = delta @ S_inv as 128x128 stationary (delta^T tiles) x [128, 512]
    moving (S) bf16 matmuls accumulated f32 in PSUM over 16 K-blocks.
  - q[b] = rowsum(delta_bf16 * Y) fused in one DVE tensor_tensor_reduce per
    (b_tile, e_chunk), chain-accumulated into q_all[:, b_tile].
  - Per-core output: q_out [128, 8] f32 (q for its 1024 rows).
  - Host: concat shards, sqrt, mean  (exact f64 host math, cast to f32).

Numerics: bf16 matmul with f32 accumulation gives ~5e-5 relative error on the
final scalar (validated against f64 numpy).
"""

import numpy as np

P = 128
B_FULL, D = 8192, 2048
N_CORES = 8
B_SH = B_FULL // N_CORES  # 1024
EC = 512                  # matmul moving free dim / PSUM bank (f32)

_CACHED = {}


def _build(b_sh=B_SH, d=D, loop=1):
    import contextlib

    import concourse.tile as tile
    from concourse import bacc, mybir

    NB = b_sh // P   # batch tiles per core
    NJ = d // P      # contraction K-blocks
    NE = d // EC     # e-chunks (output columns / 512)

    # Bacc (not raw Bass): its compile() legalizes semaphore waits
    # (move_matmul_waits_to_ldweights + generate_event_semaphores) — TRN2
    # instructions can embed only ONE sync wait.
    nc = bacc.Bacc("TRN2", target_bir_lowering=False)
    f32 = mybir.dt.float32
    bf16 = mybir.dt.bfloat16

    orig = nc.dram_tensor("orig", [b_sh, d], f32, kind="ExternalInput")
    recon = nc.dram_tensor("recon", [b_sh, d], f32, kind="ExternalInput")
    s_inv = nc.dram_tensor("s_inv", [d, d], f32, kind="ExternalInput")
    q_out = nc.dram_tensor("q_out", [P, NB], f32, kind="ExternalOutput")

    with tile.TileContext(nc) as tc:
        with (
            tc.tile_pool(name="io", bufs=3) as io_pool,
            tc.tile_pool(name="sstage", bufs=8) as s_stage,
            tc.tile_pool(name="sbf", bufs=1) as s_pool,
            tc.tile_pool(name="dbf", bufs=1) as d_pool,
            tc.tile_pool(name="dT", bufs=1) as dT_pool,
            tc.tile_pool(name="scr", bufs=2) as scr_pool,
            tc.tile_pool(name="qp", bufs=1) as q_pool,
            tc.tile_pool(name="psum", bufs=8, space="PSUM") as psum_pool,
            tc.For_i(0, loop, 1) if loop > 1 else contextlib.nullcontext(),
        ):
            q_all = q_pool.tile([P, NB], f32, name="q_all", tag="q_all")
            q_part = q_pool.tile([P, NB, NE], f32, name="q_part", tag="q_part")
            delta_bf = [None] * NB
            deltaT = [None] * NB
            s_bf = [[None] * NE for _ in range(NJ)]

            def emit_delta(t):
                # delta pipeline for batch tile t.
                # Plain loads go on the ACT (scalar) HWDGE queue so the SP
                # queue carries only transposes: a transpose waits on the DVE
                # subtract, and an in-order DMA queue would stall every later
                # load behind that wait.
                o_t = io_pool.tile([P, d], f32, name=f"o_{t}", tag="o")
                nc.scalar.dma_start(o_t[:], orig[t * P:(t + 1) * P, :])
                r_t = io_pool.tile([P, d], f32, name=f"r_{t}", tag="r")
                nc.scalar.dma_start(r_t[:], recon[t * P:(t + 1) * P, :])
                db = d_pool.tile([P, d], bf16, name=f"dbf_{t}", tag=f"dbf_{t}")
                nc.vector.tensor_sub(db[:], o_t[:], r_t[:])
                dT = dT_pool.tile([P, NJ, P], bf16, name=f"dT_{t}",
                                  tag=f"dT_{t}")
                # dT[p, j, b] = db[b, j*128 + p]  (verified in CoreSim)
                nc.sync.dma_start(dT[:], db[:], transpose=True)
                delta_bf[t] = db
                deltaT[t] = dT

            def emit_s_chunk(e):
                for j in range(NJ):
                    sf = s_stage.tile([P, EC], f32, name=f"sf_{j}_{e}",
                                      tag="sf")
                    nc.sync.dma_start(
                        sf[:], s_inv[j * P:(j + 1) * P, e * EC:(e + 1) * EC])
                    sb = s_pool.tile([P, EC], bf16, name=f"s_{j}_{e}",
                                     tag=f"s_{j}_{e}")
                    nc.scalar.copy(sb[:], sf[:])
                    s_bf[j][e] = sb

            # Emission order == desired load order: delta tiles (2 MiB each)
            # and S e-chunks (4 MiB each) interleaved so loaded-deltas ≈
            # 2 × loaded-S-chunks, which maximizes ready matmul cells per
            # loaded byte. Matmul cells are emitted in data-ready "waves"
            # matching that order, so the PE never waits on far-future loads.
            if NB == 8 and NE == 4:
                load_order = [("d", 0), ("S", 0), ("d", 1), ("S", 1),
                              ("d", 2), ("d", 3), ("S", 2), ("d", 4),
                              ("d", 5), ("S", 3), ("d", 6), ("d", 7)]
            else:
                load_order = []
                for i in range(max(NB, NE)):
                    if i < NB:
                        load_order.append(("d", i))
                    if i < NE:
                        load_order.append(("S", i))
            have_d, have_s = set(), set()
            waves = []
            for kind, idx in load_order:
                if kind == "d":
                    emit_delta(idx)
                    have_d.add(idx)
                    waves.append([(idx, e) for e in sorted(have_s)])
                else:
                    emit_s_chunk(idx)
                    have_s.add(idx)
                    waves.append([(t, idx) for t in sorted(have_d)])

            def emit_cell(t, e):
                ps = psum_pool.tile([P, EC], f32, name=f"ps_{e}_{t}", tag="ps")
                for j in range(NJ):
                    nc.tensor.matmul(
                        ps[:],
                        deltaT[t][:, j, :],
                        s_bf[j][e][:],
                        start=(j == 0),
                        stop=(j == NJ - 1),
                    )
                return ps

            for wave in waves:
                for (t, e) in wave:
                    ps = emit_cell(t, e)
                    # q-partial: product then row-reduce (two plain DVE ops;
                    # tensor_tensor_reduce faults the device on this runtime)
                    scr = scr_pool.tile([P, EC], f32, name=f"scr_{e}_{t}",
                                        tag="scr")
                    nc.vector.tensor_tensor(
                        scr[:], ps[:], delta_bf[t][:, e * EC:(e + 1) * EC],
                        mybir.AluOpType.mult)
                    nc.vector.tensor_reduce(
                        out=q_part[:, t, e:e + 1], in_=scr[:],
                        axis=mybir.AxisListType.X, op=mybir.AluOpType.add)

            nc.vector.tensor_reduce(out=q_all[:, :, None], in_=q_part[:],
                                    axis=mybir.AxisListType.X,
                                    op=mybir.AluOpType.add)
            nc.sync.dma_start(q_out[:], q_all[:])

    nc.compile()
    return nc


def _get_nc():
    if "nc" not in _CACHED:
        _CACHED["nc"] = _build()
    return _CACHED["nc"]


def kernel(original: np.ndarray, reconstruction: np.ndarray,
           S_inv: np.ndarray) -> np.ndarray:
    from concourse import bass_utils

    nc = _get_nc()
    s_full = np.ascontiguousarray(np.asarray(S_inv, dtype=np.float32))
    in_maps = []
    for i in range(N_CORES):
        sl = slice(i * B_SH, (i + 1) * B_SH)
        in_maps.append({
            "orig": np.ascontiguousarray(np.asarray(original[sl], np.float32)),
            "recon": np.ascontiguousarray(
                np.asarray(reconstruction[sl], np.float32)),
            "s_inv": s_full,
        })

    res = bass_utils.run_bass_kernel_spmd(
        nc, in_maps, core_ids=list(range(N_CORES)),
        trace=_CACHED.get("trace", False),
    )
    _CACHED["last_results"] = res

    q = np.concatenate(
        [np.asarray(r["q_out"]).T.reshape(-1) for r in res.results])
    out = np.sqrt(q.astype(np.float64)).mean()
    return np.float32(out)
